# revision 22
# baseline (speedup 1.0000x reference)
"""Multi-head self-attention (N=2048, DIM=1024, NH=16, DK=64) on 8 trn2 cores.

Head-parallel sharding: core c computes heads 2c and 2c+1.
Per core: Q/K/V projections for its 128 head-dims, scores in [m, n] layout
(row-packed K=64 matmuls, both heads concurrent in the PE array), exp on ACT,
att = [V | 1]^T @ E accumulated over m-tiles (ones column yields the softmax
denominator as row 64), then transpose back to [n, d] via PE and divide.
"""

import sys
from contextlib import ExitStack

import numpy as np

for _p in ("/opt/trn_rl_repo", "/root/.axon_site/_ro/trn_rl_repo"):
    if _p not in sys.path:
        sys.path.insert(0, _p)

import ml_dtypes  # noqa: E402

import concourse.bass as bass  # noqa: E402
import concourse.bacc as bacc  # noqa: E402
import concourse.mybir as mybir  # noqa: E402
import concourse.tile as tile  # noqa: E402
from concourse.bass import ds, ts  # noqa: E402
from concourse.bass_utils import run_bass_kernel_spmd  # noqa: E402
from concourse.masks import make_identity  # noqa: E402

N = 2048
DIM = 1024
NH = 16
DK = 64
NCORES = 8
J = 128          # head dims per core (2 heads x 64)
KT = DIM // 128  # 8 contraction tiles
MT = N // 128    # 16 m-tiles
P = 128

F32 = mybir.dt.float32
BF16 = mybir.dt.bfloat16
EXP = mybir.ActivationFunctionType.Exp

_NC_CACHE = {}


def build_nc():
    nc = bacc.Bacc("TRN2", target_bir_lowering=False, debug=False)

    # x host-packed: element (p, ((h*KT)+k)*1024 + n) = x[h*1024+n, k*128+p]
    x_d = nc.dram_tensor("xt", [P, 2 * KT * 1024], BF16, kind="ExternalInput")
    # weights host-packed: element (p, k*J+j) = W^T[k*128+p, j]
    wq_d = nc.dram_tensor("wqt", [P, KT * J], BF16, kind="ExternalInput")
    wk_d = nc.dram_tensor("wkt", [P, KT * J], BF16, kind="ExternalInput")
    wv_d = nc.dram_tensor("wvt", [P, KT * J], BF16, kind="ExternalInput")
    out_d = nc.dram_tensor("out", [N, J], F32, kind="ExternalOutput")

    with tile.TileContext(nc) as tc, ExitStack() as ctx:
        pers = ctx.enter_context(tc.tile_pool(name="pers", bufs=1))
        etp = ctx.enter_context(tc.tile_pool(name="et", bufs=8))
        vnp = ctx.enter_context(tc.tile_pool(name="vn", bufs=6))
        osbp = ctx.enter_context(tc.tile_pool(name="osb", bufs=8))
        rcp = ctx.enter_context(tc.tile_pool(name="rc", bufs=8))
        outp = ctx.enter_context(tc.tile_pool(name="outp", bufs=4))
        stp = ctx.enter_context(
            tc.tile_pool(name="stp", bufs=2, space=bass.MemorySpace.PSUM)
        )
        opp = ctx.enter_context(
            tc.tile_pool(name="opp", bufs=4, space=bass.MemorySpace.PSUM)
        )

        # ---- persistent SBUF tensors
        x_sb = pers.tile([P, 2, KT, 1024], BF16, tag="x")
        wq_sb = pers.tile([P, KT, J], BF16, tag="wq")
        wk_sb = pers.tile([P, KT, J], BF16, tag="wk")
        wv_sb = pers.tile([P, KT, J], BF16, tag="wv")
        qt_sb = pers.tile([P, N], BF16, tag="qt")
        kt_sb = pers.tile([P, N], BF16, tag="kt")
        vt_sb = pers.tile([P, N], BF16, tag="vt")
        vp_sb = pers.tile([P, MT, 2, DK + 1], BF16, tag="vp")
        ident = pers.tile([P, P], F32, tag="ident")
        wu_i = pers.tile([1, 1], F32, tag="wui")
        wu_o = pers.tile([1, 1], F32, tag="wuo")

        # ---- ACT exp-table warmup (overlaps the input DMA)
        nc.gpsimd.memset(wu_i[:, :], 0.0)
        nc.scalar.activation(wu_o[:, :], wu_i[:, :], EXP)

        # ones column for the attention matmul (denominator trick)
        nc.gpsimd.memset(vp_sb[:, :, :, :], 1.0)
        make_identity(nc, ident[:, :])

        # ---- input DMAs (all contiguous thanks to host packing)
        for w_sb, w_dd in ((wq_sb, wq_d), (wk_sb, wk_d), (wv_sb, wv_d)):
            nc.sync.dma_start(w_sb[:, :, :], w_dd[:, :])
        for h in range(2):
            nc.sync.dma_start(
                x_sb[:, h, :, :], x_d[:, ds(h * KT * 1024, KT * 1024)]
            )

        def project(dst_sb, w_sb, n0):
            """dst_sb[:, n0:n0+512] = (w^T x^T) slice, accumulated over k."""
            h, off = n0 // 1024, n0 % 1024
            ps = stp.tile([P, 512], F32, tag="st", name=f"pj_{id(w_sb)}_{n0}")
            for k in range(KT):
                nc.tensor.matmul(
                    ps[:, :],
                    w_sb[:, k, :],
                    x_sb[:, h, k, ds(off, 512)],
                    start=(k == 0),
                    stop=(k == KT - 1),
                )
            nc.vector.tensor_copy(dst_sb[:, ds(n0, 512)], ps[:, :])

        def vprep(i):
            """Build V' tiles for m-tile i: transpose Vt block, split heads."""
            vn = vnp.tile([P, P], BF16, tag="vn", name=f"vn{i}")
            nc.sync.dma_start_transpose(vn[:, :], vt_sb[:, ts(i, P)])
            nc.vector.tensor_copy(vp_sb[:, i, 0, 0:DK], vn[:, 0:DK])
            nc.vector.tensor_copy(vp_sb[:, i, 1, 0:DK], vn[:, DK:2 * DK])

        def attn_iter(i, j, nb, o_ps):
            """One (m-tile, n-chunk) step: scores both heads, exp, att both heads."""
            n0 = nb * 1024 + j * 512
            st = stp.tile([P, 1024], F32, tag="st", name=f"st{nb}_{i}_{j}")
            # h0 in rows 0-63 of the PE array, h1 in rows 64-127 (concurrent)
            nc.tensor.matmul(
                st[:, 0:512],
                kt_sb[0:DK, ts(i, P)],
                qt_sb[0:DK, ds(n0, 512)],
                start=True, stop=True,
                tile_position=(0, 0),
            )
            nc.tensor.matmul(
                st[:, 512:1024],
                kt_sb[DK:2 * DK, ts(i, P)],
                qt_sb[DK:2 * DK, ds(n0, 512)],
                start=True, stop=True,
                tile_position=(64, 0),
            )
            et = etp.tile([P, 1024], BF16, tag="et", name=f"et{nb}_{i}_{j}")
            nc.scalar.activation(et[:, :], st[:, :], EXP)
            for h in range(2):
                nc.tensor.matmul(
                    o_ps[h][j][:, :],
                    vp_sb[:, i, h, :],
                    et[:, ds(h * 512, 512)],
                    start=(i == 0),
                    stop=(i == MT - 1),
                )

        def fin_copy(nb, o_ps, osb, tail):
            """PSUM -> SBUF staging; releases the O' accumulators quickly."""
            for j in range(2):
                for h in range(2):
                    osb[j][h] = osbp.tile(
                        [DK + 1, 512], F32, tag="osb", name=f"osb{nb}_{j}_{h}"
                    )
                    if tail:
                        nc.scalar.copy(osb[j][h][:, :], o_ps[h][j][:, :])
                    else:
                        nc.vector.tensor_copy(osb[j][h][:, :], o_ps[h][j][:, :])

        def fin_transform(nb, osb, j, c, tail):
            """One 128-row output tile: PE transpose, reciprocal, scale, DMA."""
            ob = outp.tile([P, P], F32, tag="ob", name=f"ob{nb}_{j}_{c}")
            for h in range(2):
                tt = stp.tile([P, DK + 1], F32, tag="st",
                              name=f"tt{nb}_{j}_{c}_{h}")
                nc.tensor.transpose(
                    tt[:, :], osb[j][h][:, ts(c, P)], ident[0:DK + 1, 0:DK + 1]
                )
                rcr = rcp.tile([P, 1], F32, tag="rcr", name=f"rc{nb}_{j}_{c}_{h}")
                nc.vector.reciprocal(rcr[:, :], tt[:, DK:DK + 1])
                if tail:
                    nc.scalar.mul(ob[:, ds(h * DK, DK)], tt[:, 0:DK], rcr[:, :])
                else:
                    nc.vector.tensor_scalar_mul(
                        ob[:, ds(h * DK, DK)], tt[:, 0:DK], rcr[:, :]
                    )
            nc.sync.dma_start(out_d[ds(nb * 1024 + j * 512 + c * P, P), :], ob[:, :])

        # ---- lead-in projections: just enough for the first score matmuls
        project(qt_sb, wq_sb, 0)
        project(kt_sb, wk_sb, 0)
        project(qt_sb, wq_sb, 512)
        project(kt_sb, wk_sb, 512)

        # remaining projection chunks + V'-preps, spread one per m-tile so the
        # PSUM staging slots are never hogged while ACT streams exps
        inserts = {
            0: [("pv", 0), ("vp", 0)],
            1: [("pv", 512), ("vp", 4)],
            2: [("pk", 1024)],
            3: [("pv", 1024), ("vp", 8)],
            4: [("pk", 1536)],
            5: [("pv", 1536), ("vp", 12)],
            7: [("pq", 1024)],
            9: [("pq", 1536)],
        }

        o0 = [[opp.tile([DK + 1, 512], F32, tag="o", name=f"o0_{h}_{j}")
               for j in range(2)] for h in range(2)]  # [h][j]
        for i in range(MT):
            for kind, a in inserts.get(i, []):
                if kind == "pq":
                    project(qt_sb, wq_sb, a)
                elif kind == "pk":
                    project(kt_sb, wk_sb, a)
                elif kind == "pv":
                    project(vt_sb, wv_sb, a)
                else:
                    for ii in range(a, a + 4):
                        vprep(ii)
            for j in range(2):
                attn_iter(i, j, 0, o0)

        osb0 = [[None, None], [None, None]]
        fin_copy(0, o0, osb0, tail=False)

        # ---- block 1 (queries n 1024..2047); block-0 transforms interleave
        o1 = [[opp.tile([DK + 1, 512], F32, tag="o", name=f"o1_{h}_{j}")
               for j in range(2)] for h in range(2)]
        fin0_units = [(j, c) for j in range(2) for c in range(4)]
        for i in range(MT):
            for j in range(2):
                attn_iter(i, j, 1, o1)
            if 1 <= i <= 4:
                for (fj, fc) in fin0_units[(i - 1) * 2:(i - 1) * 2 + 2]:
                    fin_transform(0, osb0, fj, fc, tail=False)

        osb1 = [[None, None], [None, None]]
        fin_copy(1, o1, osb1, tail=True)
        for (fj, fc) in fin0_units:
            fin_transform(1, osb1, fj, fc, tail=True)

    nc.finalize()
    return nc


def make_in_maps(x, Wq, Wk, Wv):
    x = np.asarray(x, dtype=np.float32)
    Wq = np.asarray(Wq, dtype=np.float32)
    Wk = np.asarray(Wk, dtype=np.float32)
    Wv = np.asarray(Wv, dtype=np.float32)

    bf16 = ml_dtypes.bfloat16
    scale = 1.0 / np.sqrt(DK)
    # [P, 2*KT*1024]: element (p, (h*KT+k)*1024+n) = x[h*1024+n, k*128+p]
    xt = x.T.reshape(KT, P, 2, 1024).transpose(1, 2, 0, 3).reshape(P, 2 * KT * 1024)
    xt = np.ascontiguousarray(xt).astype(bf16)

    def pack_w(w_slice):
        # [DIM, J] -> [P, KT*J]: element (p, k*J+j) = W^T[k*P+p, j]
        wt = w_slice.T.reshape(KT, P, J).transpose(1, 0, 2).reshape(P, KT * J)
        return np.ascontiguousarray(wt).astype(bf16)

    in_maps = []
    for c in range(NCORES):
        sl = slice(c * J, (c + 1) * J)
        in_maps.append({
            "xt": xt,
            "wqt": pack_w(Wq[sl, :] * scale),
            "wkt": pack_w(Wk[sl, :]),
            "wvt": pack_w(Wv[sl, :]),
        })
    return in_maps


def kernel(x, rela, Wq, Wk, Wv):
    in_maps = make_in_maps(x, Wq, Wk, Wv)
    if "nc" not in _NC_CACHE:
        _NC_CACHE["nc"] = build_nc()
    res = run_bass_kernel_spmd(_NC_CACHE["nc"], in_maps, core_ids=list(range(NCORES)))
    out = np.concatenate([res.results[c]["out"] for c in range(NCORES)], axis=1)
    return np.ascontiguousarray(out.astype(np.float32))


if __name__ == "__main__":
    rng = np.random.default_rng(0)
    x = rng.standard_normal((N, DIM), dtype=np.float32)
    b = 1.0 / np.sqrt(DIM)
    Wq = rng.uniform(-b, b, (DIM, DIM)).astype(np.float32)
    Wk = rng.uniform(-b, b, (DIM, DIM)).astype(np.float32)
    Wv = rng.uniform(-b, b, (DIM, DIM)).astype(np.float32)
    out = kernel(x, np.zeros(1, np.float32), Wq, Wk, Wv)
    print(out.shape, out.dtype)


# revision 28
# speedup vs baseline: 1.0784x; 1.0784x over previous
"""Multi-head self-attention (N=2048, DIM=1024, NH=16, DK=64) on 8 trn2 cores.

Head-parallel sharding: core c computes heads 2c and 2c+1.
Per core: Q/K/V projections for its 128 head-dims, scores in [m, n] layout
(row-packed K=64 matmuls, both heads concurrent in the PE array), exp on ACT,
att = [V | 1]^T @ E accumulated over m-tiles (ones column yields the softmax
denominator as row 64), then transpose back to [n, d] via PE and divide.
"""

import sys
from contextlib import ExitStack

import numpy as np

for _p in ("/opt/trn_rl_repo", "/root/.axon_site/_ro/trn_rl_repo"):
    if _p not in sys.path:
        sys.path.insert(0, _p)

import ml_dtypes  # noqa: E402

import concourse.bass as bass  # noqa: E402
import concourse.bacc as bacc  # noqa: E402
import concourse.mybir as mybir  # noqa: E402
import concourse.tile as tile  # noqa: E402
from concourse.bass import ds, ts  # noqa: E402
from concourse.bass_utils import run_bass_kernel_spmd  # noqa: E402
from concourse.masks import make_identity  # noqa: E402

N = 2048
DIM = 1024
NH = 16
DK = 64
NCORES = 8
J = 128          # head dims per core (2 heads x 64)
KT = DIM // 128  # 8 contraction tiles
MT = N // 128    # 16 m-tiles
P = 128

F32 = mybir.dt.float32
BF16 = mybir.dt.bfloat16
EXP = mybir.ActivationFunctionType.Exp

_NC_CACHE = {}


def build_nc():
    nc = bacc.Bacc("TRN2", target_bir_lowering=False, debug=False)

    # x host-packed: element (p, ((h*KT)+k)*1024 + n) = x[h*1024+n, k*128+p]
    x_d = nc.dram_tensor("xt", [P, 2 * KT * 1024], BF16, kind="ExternalInput")
    # weights host-packed: element (p, k*J+j) = W^T[k*128+p, j]
    wq_d = nc.dram_tensor("wqt", [P, KT * J], BF16, kind="ExternalInput")
    wk_d = nc.dram_tensor("wkt", [P, KT * J], BF16, kind="ExternalInput")
    wv_d = nc.dram_tensor("wvt", [P, KT * J], BF16, kind="ExternalInput")
    out_d = nc.dram_tensor("out", [N, J], F32, kind="ExternalOutput")

    with tile.TileContext(nc) as tc, ExitStack() as ctx:
        pers = ctx.enter_context(tc.tile_pool(name="pers", bufs=1))
        etp = ctx.enter_context(tc.tile_pool(name="et", bufs=8))
        vnp = ctx.enter_context(tc.tile_pool(name="vn", bufs=6))
        osbp = ctx.enter_context(tc.tile_pool(name="osb", bufs=8))
        rcp = ctx.enter_context(tc.tile_pool(name="rc", bufs=8))
        outp = ctx.enter_context(tc.tile_pool(name="outp", bufs=4))
        stp = ctx.enter_context(
            tc.tile_pool(name="stp", bufs=2, space=bass.MemorySpace.PSUM)
        )
        opp = ctx.enter_context(
            tc.tile_pool(name="opp", bufs=4, space=bass.MemorySpace.PSUM)
        )

        # ---- persistent SBUF tensors
        x_sb = pers.tile([P, 2, KT, 1024], BF16, tag="x")
        wq_sb = pers.tile([P, KT, J], BF16, tag="wq")
        wk_sb = pers.tile([P, KT, J], BF16, tag="wk")
        wv_sb = pers.tile([P, KT, J], BF16, tag="wv")
        qt_sb = pers.tile([P, N], BF16, tag="qt")
        kt_sb = pers.tile([P, N], BF16, tag="kt")
        vt_sb = pers.tile([P, N], BF16, tag="vt")
        vp_sb = pers.tile([P, MT, 2, DK + 1], BF16, tag="vp")
        ident = pers.tile([P, P], F32, tag="ident")
        wu_i = pers.tile([1, 1], F32, tag="wui")
        wu_o = pers.tile([1, 1], F32, tag="wuo")

        # ---- ACT exp-table warmup (overlaps the input DMA)
        nc.gpsimd.memset(wu_i[:, :], 0.0)
        nc.scalar.activation(wu_o[:, :], wu_i[:, :], EXP)

        # ones column for the attention matmul (denominator trick)
        nc.gpsimd.memset(vp_sb[:, :, :, :], 1.0)
        make_identity(nc, ident[:, :])

        # ---- input DMAs (all contiguous thanks to host packing); first n-half
        # of x first (split by k-half so projections chase the transfer)
        for kh in range(2):
            nc.sync.dma_start(
                x_sb[:, 0, ds(kh * 4, 4), :],
                x_d[:, ds(kh * 4 * 1024, 4 * 1024)],
            )
        for w_sb, w_dd in ((wq_sb, wq_d), (wk_sb, wk_d), (wv_sb, wv_d)):
            nc.sync.dma_start(w_sb[:, :, :], w_dd[:, :])
        nc.sync.dma_start(
            x_sb[:, 1, :, :], x_d[:, ds(KT * 1024, KT * 1024)]
        )

        def project(dst_sb, w_sb, n0):
            """dst_sb[:, n0:n0+512] = (w^T x^T) slice, accumulated over k."""
            h, off = n0 // 1024, n0 % 1024
            ps = stp.tile([P, 512], F32, tag="st", name=f"pj_{id(w_sb)}_{n0}")
            for k in range(KT):
                nc.tensor.matmul(
                    ps[:, :],
                    w_sb[:, k, :],
                    x_sb[:, h, k, ds(off, 512)],
                    start=(k == 0),
                    stop=(k == KT - 1),
                )
            nc.vector.tensor_copy(dst_sb[:, ds(n0, 512)], ps[:, :])

        def vprep(i):
            """Build V' tiles for m-tile i: transpose Vt block, split heads."""
            vn = vnp.tile([P, P], BF16, tag="vn", name=f"vn{i}")
            nc.sync.dma_start_transpose(vn[:, :], vt_sb[:, ts(i, P)])
            nc.vector.tensor_copy(vp_sb[:, i, 0, 0:DK], vn[:, 0:DK])
            nc.vector.tensor_copy(vp_sb[:, i, 1, 0:DK], vn[:, DK:2 * DK])

        def scores_exp(i, j, nb):
            """Scores for both heads (row-packed, concurrent) + exp."""
            n0 = nb * 1024 + j * 512
            st = stp.tile([P, 1024], F32, tag="st", name=f"st{nb}_{i}_{j}")
            # h0 in rows 0-63 of the PE array, h1 in rows 64-127 (concurrent)
            nc.tensor.matmul(
                st[:, 0:512],
                kt_sb[0:DK, ts(i, P)],
                qt_sb[0:DK, ds(n0, 512)],
                start=True, stop=True,
                tile_position=(0, 0),
            )
            nc.tensor.matmul(
                st[:, 512:1024],
                kt_sb[DK:2 * DK, ts(i, P)],
                qt_sb[DK:2 * DK, ds(n0, 512)],
                start=True, stop=True,
                tile_position=(64, 0),
            )
            et = etp.tile([P, 1024], BF16, tag="et", name=f"et{nb}_{i}_{j}")
            nc.scalar.activation(et[:, :], st[:, :], EXP)
            return et

        def att_emit(i, j, o_ps, et):
            for h in range(2):
                nc.tensor.matmul(
                    o_ps[h][j][:, :],
                    vp_sb[:, i, h, :],
                    et[:, ds(h * 512, 512)],
                    start=(i == 0),
                    stop=(i == MT - 1),
                )

        def fin_copy(nb, o_ps, osb, tail):
            """PSUM -> SBUF staging; releases the O' accumulators quickly."""
            for j in range(2):
                for h in range(2):
                    osb[j][h] = osbp.tile(
                        [DK + 1, 512], F32, tag="osb", name=f"osb{nb}_{j}_{h}"
                    )
                    if tail and (j + h) % 2:
                        nc.scalar.copy(osb[j][h][:, :], o_ps[h][j][:, :])
                    else:
                        nc.vector.tensor_copy(osb[j][h][:, :], o_ps[h][j][:, :])

        def fin_transform(nb, osb, j, c, alt):
            """One 128-row output tile: PE transpose, reciprocal, scale, DMA.

            alt alternates the scale-multiply between ACT and DVE so the two
            engines drain the tail in parallel.
            """
            ob = outp.tile([P, P], F32, tag="ob", name=f"ob{nb}_{j}_{c}")
            for h in range(2):
                tt = stp.tile([P, DK + 1], F32, tag="st",
                              name=f"tt{nb}_{j}_{c}_{h}")
                nc.tensor.transpose(
                    tt[:, :], osb[j][h][:, ts(c, P)], ident[0:DK + 1, 0:DK + 1]
                )
                rcr = rcp.tile([P, 1], F32, tag="rcr", name=f"rc{nb}_{j}_{c}_{h}")
                nc.vector.reciprocal(rcr[:, :], tt[:, DK:DK + 1])
                if (alt + h) % 2:
                    nc.scalar.mul(ob[:, ds(h * DK, DK)], tt[:, 0:DK], rcr[:, :])
                else:
                    nc.vector.tensor_scalar_mul(
                        ob[:, ds(h * DK, DK)], tt[:, 0:DK], rcr[:, :]
                    )
            nc.sync.dma_start(out_d[ds(nb * 1024 + j * 512 + c * P, P), :], ob[:, :])

        # ---- lead-in projections: just enough for the first score matmuls
        project(qt_sb, wq_sb, 0)
        project(kt_sb, wk_sb, 0)
        project(qt_sb, wq_sb, 512)
        project(kt_sb, wk_sb, 512)

        # remaining projection chunks + V'-preps, spread one per m-tile so the
        # PSUM staging slots are never hogged while ACT streams exps
        inserts = {
            0: [("pv", 0), ("vp", 0)],
            1: [("pv", 512), ("vp", 4)],
            2: [("pk", 1024)],
            3: [("pv", 1024), ("vp", 8)],
            4: [("pk", 1536)],
            5: [("pv", 1536), ("vp", 12)],
            7: [("pq", 1024)],
            9: [("pq", 1536)],
        }

        def do_insert(kind, a):
            if kind == "pq":
                project(qt_sb, wq_sb, a)
            elif kind == "pk":
                project(kt_sb, wk_sb, a)
            elif kind == "pv":
                project(vt_sb, wv_sb, a)
            else:
                for ii in range(a, a + 4):
                    vprep(ii)

        o0 = [[opp.tile([DK + 1, 512], F32, tag="o", name=f"o0_{h}_{j}")
               for j in range(2)] for h in range(2)]  # [h][j]
        for i in range(MT):
            # inserts sit between scores (which feed ACT asap) and att (which
            # waits on the exp anyway) — PE does projection work in that window
            ets = [scores_exp(i, j, 0) for j in range(2)]
            for kind, a in inserts.get(i, []):
                do_insert(kind, a)
            for j in range(2):
                att_emit(i, j, o0, ets[j])

        osb0 = [[None, None], [None, None]]
        fin_copy(0, o0, osb0, tail=False)

        # ---- block 1 (queries n 1024..2047)
        o1 = [[opp.tile([DK + 1, 512], F32, tag="o", name=f"o1_{h}_{j}")
               for j in range(2)] for h in range(2)]
        for i in range(MT):
            ets = [scores_exp(i, j, 1) for j in range(2)]
            for j in range(2):
                att_emit(i, j, o1, ets[j])

        # ---- tail: both blocks' transforms drain across PE/DVE/ACT
        osb1 = [[None, None], [None, None]]
        fin_copy(1, o1, osb1, tail=True)
        alt = 0
        for (fj, fc) in [(j, c) for j in range(2) for c in range(4)]:
            fin_transform(0, osb0, fj, fc, alt)
            fin_transform(1, osb1, fj, fc, alt + 1)
            alt += 1

    nc.finalize()
    return nc


def make_in_maps(x, Wq, Wk, Wv):
    x = np.asarray(x, dtype=np.float32)
    Wq = np.asarray(Wq, dtype=np.float32)
    Wk = np.asarray(Wk, dtype=np.float32)
    Wv = np.asarray(Wv, dtype=np.float32)

    bf16 = ml_dtypes.bfloat16
    scale = 1.0 / np.sqrt(DK)
    # [P, 2*KT*1024]: element (p, (h*KT+k)*1024+n) = x[h*1024+n, k*128+p]
    xt = x.T.reshape(KT, P, 2, 1024).transpose(1, 2, 0, 3).reshape(P, 2 * KT * 1024)
    xt = np.ascontiguousarray(xt).astype(bf16)

    def pack_w(w_slice):
        # [DIM, J] -> [P, KT*J]: element (p, k*J+j) = W^T[k*P+p, j]
        wt = w_slice.T.reshape(KT, P, J).transpose(1, 0, 2).reshape(P, KT * J)
        return np.ascontiguousarray(wt).astype(bf16)

    in_maps = []
    for c in range(NCORES):
        sl = slice(c * J, (c + 1) * J)
        in_maps.append({
            "xt": xt,
            "wqt": pack_w(Wq[sl, :] * scale),
            "wkt": pack_w(Wk[sl, :]),
            "wvt": pack_w(Wv[sl, :]),
        })
    return in_maps


def kernel(x, rela, Wq, Wk, Wv):
    in_maps = make_in_maps(x, Wq, Wk, Wv)
    if "nc" not in _NC_CACHE:
        _NC_CACHE["nc"] = build_nc()
    res = run_bass_kernel_spmd(_NC_CACHE["nc"], in_maps, core_ids=list(range(NCORES)))
    out = np.concatenate([res.results[c]["out"] for c in range(NCORES)], axis=1)
    return np.ascontiguousarray(out.astype(np.float32))


if __name__ == "__main__":
    rng = np.random.default_rng(0)
    x = rng.standard_normal((N, DIM), dtype=np.float32)
    b = 1.0 / np.sqrt(DIM)
    Wq = rng.uniform(-b, b, (DIM, DIM)).astype(np.float32)
    Wk = rng.uniform(-b, b, (DIM, DIM)).astype(np.float32)
    Wv = rng.uniform(-b, b, (DIM, DIM)).astype(np.float32)
    out = kernel(x, np.zeros(1, np.float32), Wq, Wk, Wv)
    print(out.shape, out.dtype)


# revision 31
# speedup vs baseline: 1.1044x; 1.0241x over previous
"""Multi-head self-attention (N=2048, DIM=1024, NH=16, DK=64) on 8 trn2 cores.

Head-parallel sharding: core c computes heads 2c and 2c+1.
Per core: Q/K/V projections for its 128 head-dims, scores in [m, n] layout
(row-packed K=64 matmuls, both heads concurrent in the PE array), exp on ACT,
att = [V | 1]^T @ E accumulated over m-tiles (ones column yields the softmax
denominator as row 64), then transpose back to [n, d] via PE and divide.
"""

import sys
from contextlib import ExitStack

import numpy as np

for _p in ("/opt/trn_rl_repo", "/root/.axon_site/_ro/trn_rl_repo"):
    if _p not in sys.path:
        sys.path.insert(0, _p)

import ml_dtypes  # noqa: E402

import concourse.bass as bass  # noqa: E402
import concourse.bacc as bacc  # noqa: E402
import concourse.mybir as mybir  # noqa: E402
import concourse.tile as tile  # noqa: E402
from concourse.bass import ds, ts  # noqa: E402
from concourse.bass_utils import run_bass_kernel_spmd  # noqa: E402
from concourse.masks import make_identity  # noqa: E402

N = 2048
DIM = 1024
NH = 16
DK = 64
NCORES = 8
J = 128          # head dims per core (2 heads x 64)
KT = DIM // 128  # 8 contraction tiles
MT = N // 128    # 16 m-tiles
P = 128

F32 = mybir.dt.float32
BF16 = mybir.dt.bfloat16
EXP = mybir.ActivationFunctionType.Exp

_NC_CACHE = {}


def build_nc():
    nc = bacc.Bacc("TRN2", target_bir_lowering=False, debug=False)

    # x host-packed: element (p, ((h*KT)+k)*1024 + n) = x[h*1024+n, k*128+p]
    x_d = nc.dram_tensor("xt", [P, 2 * KT * 1024], BF16, kind="ExternalInput")
    # weights host-packed: element (p, k*J+j) = W^T[k*128+p, j]
    wq_d = nc.dram_tensor("wqt", [P, KT * J], BF16, kind="ExternalInput")
    wk_d = nc.dram_tensor("wkt", [P, KT * J], BF16, kind="ExternalInput")
    wv_d = nc.dram_tensor("wvt", [P, KT * J], BF16, kind="ExternalInput")
    out_d = nc.dram_tensor("out", [N, J], F32, kind="ExternalOutput")

    with tile.TileContext(nc) as tc, ExitStack() as ctx:
        pers = ctx.enter_context(tc.tile_pool(name="pers", bufs=1))
        etp = ctx.enter_context(tc.tile_pool(name="et", bufs=8))
        vnp = ctx.enter_context(tc.tile_pool(name="vn", bufs=6))
        osbp = ctx.enter_context(tc.tile_pool(name="osb", bufs=8))
        rcp = ctx.enter_context(tc.tile_pool(name="rc", bufs=8))
        outp = ctx.enter_context(tc.tile_pool(name="outp", bufs=4))
        stp = ctx.enter_context(
            tc.tile_pool(name="stp", bufs=2, space=bass.MemorySpace.PSUM)
        )
        opp = ctx.enter_context(
            tc.tile_pool(name="opp", bufs=4, space=bass.MemorySpace.PSUM)
        )

        # ---- persistent SBUF tensors
        x_sb = pers.tile([P, 2, KT, 1024], BF16, tag="x")
        wq_sb = pers.tile([P, KT, J], BF16, tag="wq")
        wk_sb = pers.tile([P, KT, J], BF16, tag="wk")
        wv_sb = pers.tile([P, KT, J], BF16, tag="wv")
        qt_sb = pers.tile([P, N], BF16, tag="qt")
        kt_sb = pers.tile([P, N], BF16, tag="kt")
        vt_sb = pers.tile([P, N], BF16, tag="vt")
        vp_sb = pers.tile([P, MT, 2, DK + 1], BF16, tag="vp")
        ident = pers.tile([P, P], F32, tag="ident")
        wu_i = pers.tile([1, 1], F32, tag="wui")
        wu_o = pers.tile([1, 1], F32, tag="wuo")

        # ---- ACT exp-table warmup (overlaps the input DMA)
        nc.gpsimd.memset(wu_i[:, :], 0.0)
        nc.scalar.activation(wu_o[:, :], wu_i[:, :], EXP)

        # ones column for the attention matmul (denominator trick)
        nc.gpsimd.memset(vp_sb[:, :, :, :], 1.0)
        make_identity(nc, ident[:, :])

        # ---- input DMAs (all contiguous thanks to host packing); first n-half
        # of x per k-tile so the projection k-loop chases the transfer
        for k in range(KT):
            nc.sync.dma_start(
                x_sb[:, 0, k, :], x_d[:, ds(k * 1024, 1024)]
            )
        for w_sb, w_dd in ((wq_sb, wq_d), (wk_sb, wk_d), (wv_sb, wv_d)):
            nc.sync.dma_start(w_sb[:, :, :], w_dd[:, :])
        for kh in range(2):
            nc.sync.dma_start(
                x_sb[:, 1, ds(kh * 4, 4), :],
                x_d[:, ds(KT * 1024 + kh * 4 * 1024, 4 * 1024)],
            )

        def project(dst_sb, w_sb, n0):
            """dst_sb[:, n0:n0+512] = (w^T x^T) slice, accumulated over k."""
            h, off = n0 // 1024, n0 % 1024
            ps = stp.tile([P, 512], F32, tag="st", name=f"pj_{id(w_sb)}_{n0}")
            for k in range(KT):
                nc.tensor.matmul(
                    ps[:, :],
                    w_sb[:, k, :],
                    x_sb[:, h, k, ds(off, 512)],
                    start=(k == 0),
                    stop=(k == KT - 1),
                )
            nc.vector.tensor_copy(dst_sb[:, ds(n0, 512)], ps[:, :])

        def vprep(i):
            """Build V' tiles for m-tile i: transpose Vt block, split heads."""
            vn = vnp.tile([P, P], BF16, tag="vn", name=f"vn{i}")
            nc.sync.dma_start_transpose(vn[:, :], vt_sb[:, ts(i, P)])
            nc.gpsimd.tensor_copy(vp_sb[:, i, 0, 0:DK], vn[:, 0:DK])
            nc.gpsimd.tensor_copy(vp_sb[:, i, 1, 0:DK], vn[:, DK:2 * DK])

        def scores_exp(i, j, nb):
            """Scores for both heads (row-packed, concurrent) + exp."""
            n0 = nb * 1024 + j * 512
            st = stp.tile([P, 1024], F32, tag="st", name=f"st{nb}_{i}_{j}")
            # h0 in rows 0-63 of the PE array, h1 in rows 64-127 (concurrent)
            nc.tensor.matmul(
                st[:, 0:512],
                kt_sb[0:DK, ts(i, P)],
                qt_sb[0:DK, ds(n0, 512)],
                start=True, stop=True,
                tile_position=(0, 0),
            )
            nc.tensor.matmul(
                st[:, 512:1024],
                kt_sb[DK:2 * DK, ts(i, P)],
                qt_sb[DK:2 * DK, ds(n0, 512)],
                start=True, stop=True,
                tile_position=(64, 0),
            )
            et = etp.tile([P, 1024], BF16, tag="et", name=f"et{nb}_{i}_{j}")
            nc.scalar.activation(et[:, :], st[:, :], EXP)
            return et

        def att_emit(i, j, o_ps, et):
            for h in range(2):
                nc.tensor.matmul(
                    o_ps[h][j][:, :],
                    vp_sb[:, i, h, :],
                    et[:, ds(h * 512, 512)],
                    start=(i == 0),
                    stop=(i == MT - 1),
                )

        def fin_copy(nb, o_ps, osb, tail):
            """PSUM -> SBUF staging; releases the O' accumulators quickly."""
            for j in range(2):
                for h in range(2):
                    osb[j][h] = osbp.tile(
                        [DK + 1, 512], F32, tag="osb", name=f"osb{nb}_{j}_{h}"
                    )
                    if tail and (j + h) % 2:
                        nc.scalar.copy(osb[j][h][:, :], o_ps[h][j][:, :])
                    else:
                        nc.vector.tensor_copy(osb[j][h][:, :], o_ps[h][j][:, :])

        def fin_transform(nb, osb, j, c, alt):
            """One 128-row output tile: PE transpose, reciprocal, scale, DMA.

            alt alternates the scale-multiply between ACT and DVE so the two
            engines drain the tail in parallel.
            """
            ob = outp.tile([P, P], F32, tag="ob", name=f"ob{nb}_{j}_{c}")
            for h in range(2):
                tt = stp.tile([P, DK + 1], F32, tag="st",
                              name=f"tt{nb}_{j}_{c}_{h}")
                nc.tensor.transpose(
                    tt[:, :], osb[j][h][:, ts(c, P)], ident[0:DK + 1, 0:DK + 1]
                )
                rcr = rcp.tile([P, 1], F32, tag="rcr", name=f"rc{nb}_{j}_{c}_{h}")
                nc.vector.reciprocal(rcr[:, :], tt[:, DK:DK + 1])
                if (alt + h) % 2:
                    nc.scalar.mul(ob[:, ds(h * DK, DK)], tt[:, 0:DK], rcr[:, :])
                else:
                    nc.vector.tensor_scalar_mul(
                        ob[:, ds(h * DK, DK)], tt[:, 0:DK], rcr[:, :]
                    )
            nc.gpsimd.dma_start(
                out_d[ds(nb * 1024 + j * 512 + c * P, P), :], ob[:, :]
            )

        # ---- lead-in projections: just enough for the first score matmuls
        project(qt_sb, wq_sb, 0)
        project(kt_sb, wk_sb, 0)
        project(qt_sb, wq_sb, 512)
        project(kt_sb, wk_sb, 512)

        # remaining projection chunks + V'-preps, spread one per m-tile so the
        # PSUM staging slots are never hogged while ACT streams exps
        inserts = {
            0: [("pv", 0), ("vp", 0)],
            1: [("pv", 512), ("vp", 4)],
            2: [("pk", 1024)],
            3: [("pv", 1024), ("vp", 8)],
            4: [("pk", 1536)],
            5: [("pv", 1536), ("vp", 12)],
            7: [("pq", 1024)],
            9: [("pq", 1536)],
        }

        def do_insert(kind, a):
            if kind == "pq":
                project(qt_sb, wq_sb, a)
            elif kind == "pk":
                project(kt_sb, wk_sb, a)
            elif kind == "pv":
                project(vt_sb, wv_sb, a)
            else:
                for ii in range(a, a + 4):
                    vprep(ii)

        o0 = [[opp.tile([DK + 1, 512], F32, tag="o", name=f"o0_{h}_{j}")
               for j in range(2)] for h in range(2)]  # [h][j]
        for i in range(MT):
            # inserts sit between scores (which feed ACT asap) and att (which
            # waits on the exp anyway) — PE does projection work in that window
            ets = [scores_exp(i, j, 0) for j in range(2)]
            for kind, a in inserts.get(i, []):
                do_insert(kind, a)
            for j in range(2):
                att_emit(i, j, o0, ets[j])

        osb0 = [[None, None], [None, None]]
        fin_copy(0, o0, osb0, tail=False)

        # ---- block 1 (queries n 1024..2047)
        o1 = [[opp.tile([DK + 1, 512], F32, tag="o", name=f"o1_{h}_{j}")
               for j in range(2)] for h in range(2)]
        for i in range(MT):
            ets = [scores_exp(i, j, 1) for j in range(2)]
            for j in range(2):
                att_emit(i, j, o1, ets[j])

        # ---- tail: both blocks' transforms drain across PE/DVE/ACT
        osb1 = [[None, None], [None, None]]
        fin_copy(1, o1, osb1, tail=True)
        alt = 0
        for (fj, fc) in [(j, c) for j in range(2) for c in range(4)]:
            fin_transform(0, osb0, fj, fc, alt)
            fin_transform(1, osb1, fj, fc, alt + 1)
            alt += 1

    nc.finalize()
    return nc


def make_in_maps(x, Wq, Wk, Wv):
    x = np.asarray(x, dtype=np.float32)
    Wq = np.asarray(Wq, dtype=np.float32)
    Wk = np.asarray(Wk, dtype=np.float32)
    Wv = np.asarray(Wv, dtype=np.float32)

    bf16 = ml_dtypes.bfloat16
    scale = 1.0 / np.sqrt(DK)
    # [P, 2*KT*1024]: element (p, (h*KT+k)*1024+n) = x[h*1024+n, k*128+p]
    xt = x.T.reshape(KT, P, 2, 1024).transpose(1, 2, 0, 3).reshape(P, 2 * KT * 1024)
    xt = np.ascontiguousarray(xt).astype(bf16)

    def pack_w(w_slice):
        # [DIM, J] -> [P, KT*J]: element (p, k*J+j) = W^T[k*P+p, j]
        wt = w_slice.T.reshape(KT, P, J).transpose(1, 0, 2).reshape(P, KT * J)
        return np.ascontiguousarray(wt).astype(bf16)

    in_maps = []
    for c in range(NCORES):
        sl = slice(c * J, (c + 1) * J)
        in_maps.append({
            "xt": xt,
            "wqt": pack_w(Wq[sl, :] * scale),
            "wkt": pack_w(Wk[sl, :]),
            "wvt": pack_w(Wv[sl, :]),
        })
    return in_maps


def kernel(x, rela, Wq, Wk, Wv):
    in_maps = make_in_maps(x, Wq, Wk, Wv)
    if "nc" not in _NC_CACHE:
        _NC_CACHE["nc"] = build_nc()
    res = run_bass_kernel_spmd(_NC_CACHE["nc"], in_maps, core_ids=list(range(NCORES)))
    out = np.concatenate([res.results[c]["out"] for c in range(NCORES)], axis=1)
    return np.ascontiguousarray(out.astype(np.float32))


if __name__ == "__main__":
    rng = np.random.default_rng(0)
    x = rng.standard_normal((N, DIM), dtype=np.float32)
    b = 1.0 / np.sqrt(DIM)
    Wq = rng.uniform(-b, b, (DIM, DIM)).astype(np.float32)
    Wk = rng.uniform(-b, b, (DIM, DIM)).astype(np.float32)
    Wv = rng.uniform(-b, b, (DIM, DIM)).astype(np.float32)
    out = kernel(x, np.zeros(1, np.float32), Wq, Wk, Wv)
    print(out.shape, out.dtype)


# revision 36
# speedup vs baseline: 1.2609x; 1.1417x over previous
"""Multi-head self-attention (N=2048, DIM=1024, NH=16, DK=64) on 8 trn2 cores.

Head-parallel sharding: core c computes heads 2c and 2c+1.
Per core: Q/K/V projections for its 128 head-dims, scores in [m, n] layout
(row-packed K=64 matmuls, both heads concurrent in the PE array), exp on ACT,
att = [V | 1]^T @ E accumulated over m-tiles (ones column yields the softmax
denominator as row 64), then transpose back to [n, d] via PE and divide.
"""

import sys
from contextlib import ExitStack

import numpy as np

for _p in ("/opt/trn_rl_repo", "/root/.axon_site/_ro/trn_rl_repo"):
    if _p not in sys.path:
        sys.path.insert(0, _p)

import ml_dtypes  # noqa: E402

import concourse.bass as bass  # noqa: E402
import concourse.bacc as bacc  # noqa: E402
import concourse.mybir as mybir  # noqa: E402
import concourse.tile as tile  # noqa: E402
from concourse.bass import ds, ts  # noqa: E402
from concourse.bass_utils import run_bass_kernel_spmd  # noqa: E402
from concourse.masks import make_identity  # noqa: E402

N = 2048
DIM = 1024
NH = 16
DK = 64
NCORES = 8
J = 128          # head dims per core (2 heads x 64)
KT = DIM // 128  # 8 contraction tiles
MT = N // 128    # 16 m-tiles
P = 128

F32 = mybir.dt.float32
BF16 = mybir.dt.bfloat16
EXP = mybir.ActivationFunctionType.Exp

_NC_CACHE = {}


def build_nc():
    nc = bacc.Bacc("TRN2", target_bir_lowering=False, debug=False)

    # x host-packed: element (p, ((h*KT)+k)*1024 + n) = x[h*1024+n, k*128+p]
    x_d = nc.dram_tensor("xt", [P, 2 * KT * 1024], BF16, kind="ExternalInput")
    # weights host-packed: element (p, k*J+j) = W^T[k*128+p, j]
    wq_d = nc.dram_tensor("wqt", [P, KT * J], BF16, kind="ExternalInput")
    wk_d = nc.dram_tensor("wkt", [P, KT * J], BF16, kind="ExternalInput")
    wv_d = nc.dram_tensor("wvt", [P, KT * J], BF16, kind="ExternalInput")
    out_d = nc.dram_tensor("out", [N, J], F32, kind="ExternalOutput")

    with tile.TileContext(nc) as tc, ExitStack() as ctx:
        pers = ctx.enter_context(tc.tile_pool(name="pers", bufs=1))
        etp = ctx.enter_context(tc.tile_pool(name="et", bufs=8))
        vnp = ctx.enter_context(tc.tile_pool(name="vn", bufs=6))
        osbp = ctx.enter_context(tc.tile_pool(name="osb", bufs=8))
        rcp = ctx.enter_context(tc.tile_pool(name="rc", bufs=8))
        outp = ctx.enter_context(tc.tile_pool(name="outp", bufs=4))
        stp = ctx.enter_context(
            tc.tile_pool(name="stp", bufs=2, space=bass.MemorySpace.PSUM)
        )
        opp = ctx.enter_context(
            tc.tile_pool(name="opp", bufs=2, space=bass.MemorySpace.PSUM)
        )
        pjp = ctx.enter_context(
            tc.tile_pool(name="pjp", bufs=1, space=bass.MemorySpace.PSUM)
        )
        ttp = ctx.enter_context(
            tc.tile_pool(name="ttp", bufs=1, space=bass.MemorySpace.PSUM)
        )

        # ---- persistent SBUF tensors
        x_sb = pers.tile([P, 2, KT, 1024], BF16, tag="x")
        wq_sb = pers.tile([P, KT, J], BF16, tag="wq")
        wk_sb = pers.tile([P, KT, J], BF16, tag="wk")
        wv_sb = pers.tile([P, KT, J], BF16, tag="wv")
        qt_sb = pers.tile([P, N], BF16, tag="qt")
        kt_sb = pers.tile([P, N], BF16, tag="kt")
        vt_sb = pers.tile([P, N], BF16, tag="vt")
        vp_sb = pers.tile([P, MT, 2, DK + 1], BF16, tag="vp")
        ident = pers.tile([P, P], F32, tag="ident")
        wu_i = pers.tile([1, 1], F32, tag="wui")
        wu_o = pers.tile([1, 1], F32, tag="wuo")

        # ---- ACT exp-table warmup (overlaps the input DMA)
        nc.gpsimd.memset(wu_i[:, :], 0.0)
        nc.scalar.activation(wu_o[:, :], wu_i[:, :], EXP)

        # ones column for the attention matmul (denominator trick)
        nc.gpsimd.memset(vp_sb[:, :, :, :], 1.0)
        make_identity(nc, ident[:, :])

        # ---- input DMAs (all contiguous thanks to host packing); first n-half
        # of x per k-tile so the projection k-loop chases the transfer
        # weights first (small; needed by the first projection), then x n-half 0
        # split in two (k-halves) issued from two engines, then x n-half 1
        for w_sb, w_dd in ((wq_sb, wq_d), (wk_sb, wk_d), (wv_sb, wv_d)):
            nc.sync.dma_start(w_sb[:, :, :], w_dd[:, :])
        nc.sync.dma_start(x_sb[:, 0, 0:4, :], x_d[:, ds(0, 4 * 1024)])
        nc.gpsimd.dma_start(x_sb[:, 0, 4:8, :], x_d[:, ds(4 * 1024, 4 * 1024)])
        nc.sync.dma_start(
            x_sb[:, 1, 0:4, :], x_d[:, ds(KT * 1024, 4 * 1024)]
        )
        nc.gpsimd.dma_start(
            x_sb[:, 1, 4:8, :], x_d[:, ds(KT * 1024 + 4 * 1024, 4 * 1024)]
        )

        _pj_alt = [0]

        def project(dst_sb, w_sb, n0):
            """dst_sb[:, n0:n0+512] = (w^T x^T) slice, accumulated over k."""
            h, off = n0 // 1024, n0 % 1024
            pool, tg = ((pjp, "pj"), (ttp, "tt"))[_pj_alt[0] % 2]
            _pj_alt[0] += 1
            ps = pool.tile([P, 512], F32, tag=tg, name=f"pj_{id(w_sb)}_{n0}")
            for k in range(KT):
                nc.tensor.matmul(
                    ps[:, :],
                    w_sb[:, k, :],
                    x_sb[:, h, k, ds(off, 512)],
                    start=(k == 0),
                    stop=(k == KT - 1),
                )
            nc.vector.tensor_copy(dst_sb[:, ds(n0, 512)], ps[:, :])

        def vprep(i):
            """Build V' tiles for m-tile i: transpose Vt block, split heads."""
            vn = vnp.tile([P, P], BF16, tag="vn", name=f"vn{i}")
            nc.sync.dma_start_transpose(vn[:, :], vt_sb[:, ts(i, P)])
            nc.gpsimd.tensor_copy(vp_sb[:, i, 0, 0:DK], vn[:, 0:DK])
            nc.gpsimd.tensor_copy(vp_sb[:, i, 1, 0:DK], vn[:, DK:2 * DK])

        def scores_exp(i, p):
            """Scores for both heads (row-packed, concurrent) + exp; pass p."""
            n0 = p * 512
            st = stp.tile([P, 1024], F32, tag="st", name=f"st{p}_{i}")
            # h0 in rows 0-63 of the PE array, h1 in rows 64-127 (concurrent)
            nc.tensor.matmul(
                st[:, 0:512],
                kt_sb[0:DK, ts(i, P)],
                qt_sb[0:DK, ds(n0, 512)],
                start=True, stop=True,
                tile_position=(0, 0),
            )
            nc.tensor.matmul(
                st[:, 512:1024],
                kt_sb[DK:2 * DK, ts(i, P)],
                qt_sb[DK:2 * DK, ds(n0, 512)],
                start=True, stop=True,
                tile_position=(64, 0),
            )
            et = etp.tile([P, 1024], BF16, tag="et", name=f"et{p}_{i}")
            nc.scalar.activation(et[:, :], st[:, :], EXP)
            return et

        def att_emit(i, o_ps, et):
            for h in range(2):
                nc.tensor.matmul(
                    o_ps[h][:, :],
                    vp_sb[:, i, h, :],
                    et[:, ds(h * 512, 512)],
                    start=(i == 0),
                    stop=(i == MT - 1),
                )

        def fin_copy(p, o_ps, osb, tail):
            """PSUM -> SBUF staging; releases the O' accumulators quickly."""
            for h in range(2):
                osb[h] = osbp.tile(
                    [DK + 1, 512], F32, tag="osb", name=f"osb{p}_{h}"
                )
                if tail and h == 1:
                    nc.scalar.copy(osb[h][:, :], o_ps[h][:, :])
                else:
                    nc.vector.tensor_copy(osb[h][:, :], o_ps[h][:, :])

        def fin_transform(p, osb, tail):
            """Transpose the pass's [65, 512] staging to [n, d], divide by the
            row-sums (batched: one reciprocal + one multiply per head), DMA."""
            ob = outp.tile([P, 4, P], F32, tag="ob", name=f"ob{p}")
            for h in range(2):
                tt = ttp.tile([P, 4, DK + 1], F32, tag="tt", name=f"tt{p}_{h}")
                for c in range(4):
                    nc.tensor.transpose(
                        tt[:, c, :], osb[h][:, ts(c, P)],
                        ident[0:DK + 1, 0:DK + 1],
                    )
                rcr = rcp.tile([P, 4], F32, tag="rcr", name=f"rc{p}_{h}")
                nc.vector.reciprocal(rcr[:, :], tt[:, :, DK])
                rb = rcr[:, :, None].broadcast_to([P, 4, DK])
                nc.vector.tensor_tensor(
                    ob[:, :, ds(h * DK, DK)], tt[:, :, 0:DK], rb,
                    op=mybir.AluOpType.mult,
                )
            nc.gpsimd.dma_start(
                out_d[ds(p * 512, 512), :].rearrange("(c q) j -> q c j", c=4),
                ob[:, :, :],
            )

        # ---- lead-in projections: just enough for the first pass to start
        project(qt_sb, wq_sb, 0)
        project(kt_sb, wk_sb, 0)

        # remaining projection chunks + V'-preps, spread across pass-0 (and the
        # next-pass Q chunks late in each pass)
        inserts = {
            0: {0: [("pv", 0), ("vp", 0)],
                1: [("pk", 512)],
                2: [("pv", 512), ("vp", 4)],
                3: [("pk", 1024)],
                5: [("pv", 1024), ("vp", 8)],
                7: [("pk", 1536)],
                9: [("pv", 1536), ("vp", 12)],
                11: [("pq", 512)]},
            1: {2: [("pq", 1024)]},
            2: {2: [("pq", 1536)]},
        }

        def do_insert(kind, a):
            if kind == "pq":
                project(qt_sb, wq_sb, a)
            elif kind == "pk":
                project(kt_sb, wk_sb, a)
            elif kind == "pv":
                project(vt_sb, wv_sb, a)
            else:
                for ii in range(a, a + 4):
                    vprep(ii)

        osbs = [[None, None] for _ in range(4)]
        for p in range(4):
            o_ps = [opp.tile([DK + 1, 512], F32, tag="o", name=f"o{p}_{h}")
                    for h in range(2)]
            pins = inserts.get(p, {})
            for i in range(MT):
                et = scores_exp(i, p)
                for kind, a in pins.get(i, []):
                    do_insert(kind, a)
                if p > 0 and i == 6:
                    # previous pass's output transform, on the dedicated slot
                    fin_transform(p - 1, osbs[p - 1], tail=False)
                att_emit(i, o_ps, et)
            fin_copy(p, o_ps, osbs[p], tail=(p == 3))
        fin_transform(3, osbs[3], tail=True)

    nc.finalize()
    return nc


def make_in_maps(x, Wq, Wk, Wv):
    x = np.asarray(x, dtype=np.float32)
    Wq = np.asarray(Wq, dtype=np.float32)
    Wk = np.asarray(Wk, dtype=np.float32)
    Wv = np.asarray(Wv, dtype=np.float32)

    bf16 = ml_dtypes.bfloat16
    scale = 1.0 / np.sqrt(DK)
    # [P, 2*KT*1024]: element (p, (h*KT+k)*1024+n) = x[h*1024+n, k*128+p]
    xt = x.T.reshape(KT, P, 2, 1024).transpose(1, 2, 0, 3).reshape(P, 2 * KT * 1024)
    xt = np.ascontiguousarray(xt).astype(bf16)

    def pack_w(w_slice):
        # [DIM, J] -> [P, KT*J]: element (p, k*J+j) = W^T[k*P+p, j]
        wt = w_slice.T.reshape(KT, P, J).transpose(1, 0, 2).reshape(P, KT * J)
        return np.ascontiguousarray(wt).astype(bf16)

    in_maps = []
    for c in range(NCORES):
        sl = slice(c * J, (c + 1) * J)
        in_maps.append({
            "xt": xt,
            "wqt": pack_w(Wq[sl, :] * scale),
            "wkt": pack_w(Wk[sl, :]),
            "wvt": pack_w(Wv[sl, :]),
        })
    return in_maps


def kernel(x, rela, Wq, Wk, Wv):
    in_maps = make_in_maps(x, Wq, Wk, Wv)
    if "nc" not in _NC_CACHE:
        _NC_CACHE["nc"] = build_nc()
    res = run_bass_kernel_spmd(_NC_CACHE["nc"], in_maps, core_ids=list(range(NCORES)))
    out = np.concatenate([res.results[c]["out"] for c in range(NCORES)], axis=1)
    return np.ascontiguousarray(out.astype(np.float32))


if __name__ == "__main__":
    rng = np.random.default_rng(0)
    x = rng.standard_normal((N, DIM), dtype=np.float32)
    b = 1.0 / np.sqrt(DIM)
    Wq = rng.uniform(-b, b, (DIM, DIM)).astype(np.float32)
    Wk = rng.uniform(-b, b, (DIM, DIM)).astype(np.float32)
    Wv = rng.uniform(-b, b, (DIM, DIM)).astype(np.float32)
    out = kernel(x, np.zeros(1, np.float32), Wq, Wk, Wv)
    print(out.shape, out.dtype)


# revision 42
# speedup vs baseline: 1.3283x; 1.0535x over previous
"""Multi-head self-attention (N=2048, DIM=1024, NH=16, DK=64) on 8 trn2 cores.

Head-parallel sharding: core c computes heads 2c and 2c+1.
Per core: Q/K/V projections for its 128 head-dims, scores in [m, n] layout
(row-packed K=64 matmuls, both heads concurrent in the PE array), exp on ACT,
att = [V | 1]^T @ E accumulated over m-tiles (ones column yields the softmax
denominator as row 64), then transpose back to [n, d] via PE and divide.
"""

import sys
from contextlib import ExitStack

import numpy as np

for _p in ("/opt/trn_rl_repo", "/root/.axon_site/_ro/trn_rl_repo"):
    if _p not in sys.path:
        sys.path.insert(0, _p)

import ml_dtypes  # noqa: E402

import concourse.bass as bass  # noqa: E402
import concourse.bacc as bacc  # noqa: E402
import concourse.mybir as mybir  # noqa: E402
import concourse.tile as tile  # noqa: E402
from concourse.bass import ds, ts  # noqa: E402
from concourse.bass_utils import run_bass_kernel_spmd  # noqa: E402
from concourse.masks import make_identity  # noqa: E402

N = 2048
DIM = 1024
NH = 16
DK = 64
NCORES = 8
J = 128          # head dims per core (2 heads x 64)
KT = DIM // 128  # 8 contraction tiles
MT = N // 128    # 16 m-tiles
P = 128

F32 = mybir.dt.float32
BF16 = mybir.dt.bfloat16
EXP = mybir.ActivationFunctionType.Exp

_NC_CACHE = {}


def build_nc():
    nc = bacc.Bacc("TRN2", target_bir_lowering=False, debug=False)

    # x host-packed by n-quarter: element (p, ((q*KT)+k)*512 + n) =
    # x[q*512+n, k*128+p] — each quarter is one contiguous 1MB DMA
    x_d = nc.dram_tensor("xt", [P, 4 * KT * 512], BF16, kind="ExternalInput")
    # weights host-packed: element (p, k*J+j) = W^T[k*128+p, j]
    wq_d = nc.dram_tensor("wqt", [P, KT * J], BF16, kind="ExternalInput")
    wk_d = nc.dram_tensor("wkt", [P, KT * J], BF16, kind="ExternalInput")
    wv_d = nc.dram_tensor("wvt", [P, KT * J], BF16, kind="ExternalInput")
    out_d = nc.dram_tensor("out", [N, J], F32, kind="ExternalOutput")

    with tile.TileContext(nc) as tc, ExitStack() as ctx:
        pers = ctx.enter_context(tc.tile_pool(name="pers", bufs=1))
        etp = ctx.enter_context(tc.tile_pool(name="et", bufs=8))
        vnp = ctx.enter_context(tc.tile_pool(name="vn", bufs=6))
        osbp = ctx.enter_context(tc.tile_pool(name="osb", bufs=8))
        rcp = ctx.enter_context(tc.tile_pool(name="rc", bufs=8))
        outp = ctx.enter_context(tc.tile_pool(name="outp", bufs=4))
        stp = ctx.enter_context(
            tc.tile_pool(name="stp", bufs=2, space=bass.MemorySpace.PSUM)
        )
        opp = ctx.enter_context(
            tc.tile_pool(name="opp", bufs=2, space=bass.MemorySpace.PSUM)
        )
        pjp = ctx.enter_context(
            tc.tile_pool(name="pjp", bufs=1, space=bass.MemorySpace.PSUM)
        )
        ttp = ctx.enter_context(
            tc.tile_pool(name="ttp", bufs=1, space=bass.MemorySpace.PSUM)
        )

        # ---- persistent SBUF tensors
        x_sb = pers.tile([P, 4, KT, 512], BF16, tag="x")
        wq_sb = pers.tile([P, KT, J], BF16, tag="wq")
        wk_sb = pers.tile([P, KT, J], BF16, tag="wk")
        wv_sb = pers.tile([P, KT, J], BF16, tag="wv")
        qt_sb = pers.tile([P, N], BF16, tag="qt")
        kt_sb = pers.tile([P, N], BF16, tag="kt")
        vt_sb = pers.tile([P, N], BF16, tag="vt")
        vp_sb = pers.tile([P, MT, 2, DK + 1], BF16, tag="vp")
        ident = pers.tile([P, P], F32, tag="ident")
        wu_i = pers.tile([1, 1], F32, tag="wui")
        wu_o = pers.tile([1, 1], F32, tag="wuo")

        # ---- ACT exp-table warmup (overlaps the input DMA)
        nc.gpsimd.memset(wu_i[:, :], 0.0)
        nc.scalar.activation(wu_o[:, :], wu_i[:, :], EXP)

        # ones column for the attention matmul (denominator trick)
        nc.gpsimd.memset(vp_sb[:, :, :, :], 1.0)
        make_identity(nc, ident[:, :])

        # ---- input DMAs (all contiguous thanks to host packing); first n-half
        # of x per k-tile so the projection k-loop chases the transfer
        # weights on gpsimd (small), x n-quarters in parallel from 4 queues so
        # the first projections start after just the first 1MB lands
        for w_sb, w_dd in ((wq_sb, wq_d), (wk_sb, wk_d), (wv_sb, wv_d)):
            nc.gpsimd.dma_start(w_sb[:, :, :], w_dd[:, :])
        qsz = KT * 512
        nc.sync.dma_start(x_sb[:, 0, :, :], x_d[:, ds(0, qsz)])
        nc.scalar.dma_start(x_sb[:, 1, :, :], x_d[:, ds(qsz, qsz)])
        nc.gpsimd.dma_start(x_sb[:, 2, :, :], x_d[:, ds(2 * qsz, qsz)])
        nc.sync.dma_start(x_sb[:, 3, :, :], x_d[:, ds(3 * qsz, qsz)])

        _pj_alt = [0]

        def project(dst_sb, w_sb, n0):
            """dst_sb[:, n0:n0+512] = (w^T x^T) slice, accumulated over k."""
            q = n0 // 512
            pool, tg = ((pjp, "pj"), (ttp, "tt"))[_pj_alt[0] % 2]
            _pj_alt[0] += 1
            ps = pool.tile([P, 512], F32, tag=tg, name=f"pj_{id(w_sb)}_{n0}")
            for k in range(KT):
                nc.tensor.matmul(
                    ps[:, :],
                    w_sb[:, k, :],
                    x_sb[:, q, k, :],
                    start=(k == 0),
                    stop=(k == KT - 1),
                )
            nc.vector.tensor_copy(dst_sb[:, ds(n0, 512)], ps[:, :])

        def vprep(i):
            """Build V' tiles for m-tile i: transpose Vt block, split heads."""
            vn = vnp.tile([P, P], BF16, tag="vn", name=f"vn{i}")
            nc.sync.dma_start_transpose(vn[:, :], vt_sb[:, ts(i, P)])
            nc.gpsimd.tensor_copy(vp_sb[:, i, 0, 0:DK], vn[:, 0:DK])
            nc.gpsimd.tensor_copy(vp_sb[:, i, 1, 0:DK], vn[:, DK:2 * DK])

        def scores_exp(i, p):
            """Scores for both heads (row-packed, concurrent) + exp; pass p."""
            n0 = p * 512
            st = stp.tile([P, 1024], F32, tag="st", name=f"st{p}_{i}")
            # h0 in rows 0-63 of the PE array, h1 in rows 64-127 (concurrent)
            nc.tensor.matmul(
                st[:, 0:512],
                kt_sb[0:DK, ts(i, P)],
                qt_sb[0:DK, ds(n0, 512)],
                start=True, stop=True,
                tile_position=(0, 0),
            )
            nc.tensor.matmul(
                st[:, 512:1024],
                kt_sb[DK:2 * DK, ts(i, P)],
                qt_sb[DK:2 * DK, ds(n0, 512)],
                start=True, stop=True,
                tile_position=(64, 0),
            )
            et = etp.tile([P, 1024], BF16, tag="et", name=f"et{p}_{i}")
            nc.scalar.activation(et[:, :], st[:, :], EXP)
            return et

        def att_emit(i, o_ps, et):
            for h in range(2):
                nc.tensor.matmul(
                    o_ps[h][:, :],
                    vp_sb[:, i, h, :],
                    et[:, ds(h * 512, 512)],
                    start=(i == 0),
                    stop=(i == MT - 1),
                )

        def fin_copy(p, o_ps, osb, tail):
            """PSUM -> SBUF staging; releases the O' accumulators quickly."""
            for h in range(2):
                osb[h] = osbp.tile(
                    [DK + 1, 512], F32, tag="osb", name=f"osb{p}_{h}"
                )
                if tail and h == 1:
                    nc.scalar.copy(osb[h][:, :], o_ps[h][:, :])
                else:
                    nc.vector.tensor_copy(osb[h][:, :], o_ps[h][:, :])

        def fin_transform(p, osb, tail):
            """Transpose the pass's [65, 512] staging to [n, d], divide by the
            row-sums (batched: one reciprocal + one multiply per head), DMA."""
            ob = outp.tile([P, 4, P], F32, tag="ob", name=f"ob{p}")
            for h in range(2):
                tt = ttp.tile([P, 4, DK + 1], F32, tag="tt", name=f"tt{p}_{h}")
                for c in range(4):
                    nc.tensor.transpose(
                        tt[:, c, :], osb[h][:, ts(c, P)],
                        ident[0:DK + 1, 0:DK + 1],
                    )
                rcr = rcp.tile([P, 4], F32, tag="rcr", name=f"rc{p}_{h}")
                nc.vector.reciprocal(rcr[:, :], tt[:, :, DK])
                rb = rcr[:, :, None].broadcast_to([P, 4, DK])
                nc.vector.tensor_tensor(
                    ob[:, :, ds(h * DK, DK)], tt[:, :, 0:DK], rb,
                    op=mybir.AluOpType.mult,
                )
            nc.gpsimd.dma_start(
                out_d[ds(p * 512, 512), :].rearrange("(c q) j -> q c j", c=4),
                ob[:, :, :],
            )

        # ---- lead-in projections: just enough for the first pass to start
        project(qt_sb, wq_sb, 0)
        project(kt_sb, wk_sb, 0)

        # remaining projection chunks + V'-preps, spread across pass-0 (and the
        # next-pass Q chunks late in each pass)
        inserts = {
            0: {0: [("pv", 0), ("vp", 0)],
                1: [("pk", 512)],
                2: [("pv", 512), ("vp", 4)],
                3: [("pk", 1024)],
                5: [("pv", 1024), ("vp", 8)],
                7: [("pk", 1536)],
                9: [("pv", 1536), ("vp", 12)],
                11: [("pq", 512)]},
            1: {2: [("pq", 1024)]},
            2: {2: [("pq", 1536)]},
        }

        def do_insert(kind, a):
            if kind == "pq":
                project(qt_sb, wq_sb, a)
            elif kind == "pk":
                project(kt_sb, wk_sb, a)
            elif kind == "pv":
                project(vt_sb, wv_sb, a)
            else:
                for ii in range(a, a + 4):
                    vprep(ii)

        osbs = [[None, None] for _ in range(4)]
        for p in range(4):
            o_ps = [opp.tile([DK + 1, 512], F32, tag="o", name=f"o{p}_{h}")
                    for h in range(2)]
            pins = inserts.get(p, {})
            for i in range(MT):
                et = scores_exp(i, p)
                for kind, a in pins.get(i, []):
                    do_insert(kind, a)
                if p > 0 and i == 6:
                    # previous pass's output transform, on the dedicated slot
                    fin_transform(p - 1, osbs[p - 1], tail=False)
                att_emit(i, o_ps, et)
            fin_copy(p, o_ps, osbs[p], tail=(p == 3))
        fin_transform(3, osbs[3], tail=True)

    nc.finalize()
    return nc


def make_in_maps(x, Wq, Wk, Wv):
    x = np.asarray(x, dtype=np.float32)
    Wq = np.asarray(Wq, dtype=np.float32)
    Wk = np.asarray(Wk, dtype=np.float32)
    Wv = np.asarray(Wv, dtype=np.float32)

    bf16 = ml_dtypes.bfloat16
    scale = 1.0 / np.sqrt(DK)
    # [P, 4*KT*512]: element (p, (q*KT+k)*512+n) = x[q*512+n, k*128+p]
    xt = x.T.reshape(KT, P, 4, 512).transpose(1, 2, 0, 3).reshape(P, 4 * KT * 512)
    xt = np.ascontiguousarray(xt).astype(bf16)

    def pack_w(w_slice):
        # [DIM, J] -> [P, KT*J]: element (p, k*J+j) = W^T[k*P+p, j]
        wt = w_slice.T.reshape(KT, P, J).transpose(1, 0, 2).reshape(P, KT * J)
        return np.ascontiguousarray(wt).astype(bf16)

    in_maps = []
    for c in range(NCORES):
        sl = slice(c * J, (c + 1) * J)
        in_maps.append({
            "xt": xt,
            "wqt": pack_w(Wq[sl, :] * scale),
            "wkt": pack_w(Wk[sl, :]),
            "wvt": pack_w(Wv[sl, :]),
        })
    return in_maps


def kernel(x, rela, Wq, Wk, Wv):
    in_maps = make_in_maps(x, Wq, Wk, Wv)
    if "nc" not in _NC_CACHE:
        _NC_CACHE["nc"] = build_nc()
    res = run_bass_kernel_spmd(_NC_CACHE["nc"], in_maps, core_ids=list(range(NCORES)))
    out = np.concatenate([res.results[c]["out"] for c in range(NCORES)], axis=1)
    return np.ascontiguousarray(out.astype(np.float32))


if __name__ == "__main__":
    rng = np.random.default_rng(0)
    x = rng.standard_normal((N, DIM), dtype=np.float32)
    b = 1.0 / np.sqrt(DIM)
    Wq = rng.uniform(-b, b, (DIM, DIM)).astype(np.float32)
    Wk = rng.uniform(-b, b, (DIM, DIM)).astype(np.float32)
    Wv = rng.uniform(-b, b, (DIM, DIM)).astype(np.float32)
    out = kernel(x, np.zeros(1, np.float32), Wq, Wk, Wv)
    print(out.shape, out.dtype)
